# revision 1
# baseline (speedup 1.0000x reference)
"""BiLSTM tagger kernel for 8 Trainium2 NeuronCores.

Strategy: data-parallel over batch (16 sequences per core, weights
replicated). Per core, the two directions of each BiLSTM layer run as
interleaved scans so gate math on ScalarE/VectorE hides under the other
scan's recurrent matmul on TensorE. All matmuls run in bf16 (fp32 matmul
is 4x slower on TRN2); PSUM accumulation stays fp32.

Recurrent step layout: stationary = h^T chunks [128,16], moving = W_hh^T
slices, psum gates [16, 2048]. gx (input projections, precomputed per
layer into HBM) is added on VectorE during the psum drain. h is
re-transposed each step with four tiny matmuls against a 16x16 identity.
Backward scans consume inputs pre-reversed per sequence length (host
permutation indices + indirect DMA); their outputs are scattered back
through the same permutation, which also writes the zero padding the
reference produces. The permutation is t -> len-1-t for t < len, else
t -> t; steps past len compute garbage that is masked to zero and cannot
contaminate earlier steps.
"""

import sys

for _p in ("/opt/trn_rl_repo",):
    if _p not in sys.path:
        sys.path.append(_p)

import numpy as np
import ml_dtypes

import concourse.bass as bass
import concourse.tile as tile
from concourse import bacc, mybir
from concourse.bass import IndirectOffsetOnAxis
from concourse.bass_utils import run_bass_kernel_spmd

F32 = mybir.dt.float32
BF16 = mybir.dt.bfloat16
I32 = mybir.dt.int32
AF = mybir.ActivationFunctionType
ALU = mybir.AluOpType

# problem sizes (full / per-core)
B, T, V, E, H, TAGS = 128, 512, 50000, 256, 512, 64
NC = 8
BL = B // NC   # 16 sequences per core
G = 4 * H      # 2048 gate width

ABLATE = set()  # dev knob: {"gxdma","gates","ring","trans","mm"}

# permutation taking pytorch gate order i,f,g,o -> i,f,o,g (sigmoid block first)
_GATE_PERM = np.concatenate([
    np.arange(0, H), np.arange(H, 2 * H), np.arange(3 * H, 4 * H),
    np.arange(2 * H, 3 * H)])


def _build(nc, Tn=T, Bl=BL, TC=2, RC=4):
    """Emit the per-core program. Tn shrinkable for dev testing."""
    ntok = Bl * Tn
    nchunk = ntok // 128
    KE = E // 128       # k-chunks for layer-1 input proj
    KH2 = 2 * H // 128  # k-chunks for layer-2 input proj / classifier
    KH = H // 128       # k-chunks for recurrent
    assert ntok % 128 == 0

    # ---- dram I/O ----
    emb = nc.dram_tensor("emb", [V, E], F32, kind="ExternalInput")
    xf_idx = nc.dram_tensor("xf_idx", [128, nchunk], I32, kind="ExternalInput")
    xb_idx = nc.dram_tensor("xb_idx", [128, nchunk], I32, kind="ExternalInput")
    rev128 = nc.dram_tensor("rev128", [128, nchunk], I32, kind="ExternalInput")
    rev16 = nc.dram_tensor("rev16", [Bl, Tn], I32, kind="ExternalInput")
    mask = nc.dram_tensor("mask", [Bl, Tn], F32, kind="ExternalInput")
    ident = nc.dram_tensor("ident", [16, 16], BF16, kind="ExternalInput")

    wih, whh, biasd = {}, {}, {}
    for s, din in (("f1", E), ("b1", E), ("f2", 2 * H), ("b2", 2 * H)):
        wih[s] = nc.dram_tensor(f"wihT_{s}", [din, G], BF16, kind="ExternalInput")
        whh[s] = nc.dram_tensor(f"whhT_{s}", [H, G], BF16, kind="ExternalInput")
        biasd[s] = nc.dram_tensor(f"bias_{s}", [128, G], F32, kind="ExternalInput")
    wcls = nc.dram_tensor("wclsT", [2 * H, TAGS], BF16, kind="ExternalInput")
    bcls = nc.dram_tensor("bcls", [TAGS, 1], F32, kind="ExternalInput")

    gx = {s: nc.dram_tensor(f"gx_{s}", [ntok, G], BF16)
          for s in ("f1", "b1", "f2", "b2")}
    # per-direction layer outputs; backward halves stay in scan order and are
    # un-reversed by the consumers' row gathers (no per-step scatters)
    hout = {s: nc.dram_tensor(f"hout_{s}", [ntok, H], BF16)
            for s in ("f1", "b1", "f2", "b2")}
    logitsT = nc.dram_tensor("logitsT", [TAGS, ntok], F32, kind="ExternalOutput")

    with tile.TileContext(nc) as tc:
        with tc.tile_pool(name="const", bufs=1) as cpool:
            def load_const(nm, shape, dt, src_ap):
                t = cpool.tile(shape, dt, name=nm, tag=nm)
                nc.gpsimd.dma_start(t[:], src_ap)
                return t

            xf_sb = load_const("xf_sb", [128, nchunk], I32, xf_idx[:])
            xb_sb = load_const("xb_sb", [128, nchunk], I32, xb_idx[:])
            rev128_sb = load_const("rev128_sb", [128, nchunk], I32, rev128[:])
            rev16_sb = load_const("rev16_sb", [Bl, Tn], I32, rev16[:])
            mask_sb = load_const("mask_sb", [Bl, Tn], F32, mask[:])
            id_sb = load_const("id_sb", [16, 16], BF16, ident[:])
            bcls_sb = load_const("bcls_sb", [TAGS, 1], F32, bcls[:])
            bias_sb = {s: load_const(f"bias_sb_{s}", [128, G], F32, biasd[s][:])
                       for s in ("f1", "b1", "f2", "b2")}
            wcls_sb = cpool.tile([128, KH2, TAGS], BF16, name="wcls_sb")
            for k in range(KH2):
                nc.gpsimd.dma_start(wcls_sb[:, k, :], wcls[128 * k:128 * (k + 1), :])

            # layer-1 input projections (inputs gathered from embedding table)
            _proj_phase(nc, tc, nchunk, KE, wih=wih, bias_sb=bias_sb, gx=gx,
                        jobs=[("f1", emb, xf_sb, True), ("b1", emb, xb_sb, True)])
            # layer-1 scans
            _scan_phase(nc, tc, Tn, Bl, TC, RC, KH,
                        scans=("f1", "b1"), whh=whh, gx=gx, hout=hout,
                        mask_sb=mask_sb, id_sb=id_sb)
            # layer-2 input projections: input token (b,t) for the fwd scan is
            # [f1h[t], s1h[rev(t)]]; for the bwd scan it is [f1h[rev(t)], s1h[t]]
            _proj_phase(nc, tc, nchunk, KH2, wih=wih, bias_sb=bias_sb, gx=gx,
                        jobs=[("f2", (hout["f1"], None, hout["b1"], rev128_sb), None, False),
                              ("b2", (hout["f1"], rev128_sb, hout["b1"], None), None, False)])
            # layer-2 scans
            _scan_phase(nc, tc, Tn, Bl, TC, RC, KH,
                        scans=("f2", "b2"), whh=whh, gx=gx, hout=hout,
                        mask_sb=mask_sb, id_sb=id_sb)

            # classifier: logits^T = W_cls @ out2^T + b_cls
            with tc.tile_pool(name="cls", bufs=3) as gp, \
                 tc.tile_pool(name="clsT", bufs=3) as gtp, \
                 tc.tile_pool(name="clsps", bufs=4, space="PSUM") as pp, \
                 tc.tile_pool(name="clso", bufs=3) as op:
                for c in range(nchunk):
                    o2 = gp.tile([128, 2 * H], BF16, tag="in")
                    nc.gpsimd.dma_start(o2[:, 0:H], hout["f2"][128 * c:128 * (c + 1), :])
                    nc.gpsimd.indirect_dma_start(
                        out=o2[:, H:2 * H], out_offset=None, in_=hout["b2"][:],
                        in_offset=IndirectOffsetOnAxis(ap=rev128_sb[:, c:c + 1], axis=0))
                    o2T = gtp.tile([128, KH2, 128], BF16, tag="inT")
                    for k in range(KH2):
                        nc.sync.dma_start_transpose(
                            o2T[:, k, :], o2[:, 128 * k:128 * (k + 1)])
                    ps = pp.tile([TAGS, 128], F32, name="clsps_t")
                    for k in range(KH2):
                        nc.tensor.matmul(ps[:], wcls_sb[:, k, :], o2T[:, k, :],
                                         start=(k == 0), stop=(k == KH2 - 1))
                    lg = op.tile([TAGS, 128], F32, tag="lg")
                    nc.scalar.activation(lg[:], ps[:], AF.Identity,
                                         bias=bcls_sb[:, 0:1])
                    nc.gpsimd.dma_start(logitsT[:, 128 * c:128 * (c + 1)], lg[:])
    return nc


def _proj_phase(nc, tc, nchunk, KD, wih, bias_sb, gx, jobs):
    """gx_s = input @ W_ih_s^T + b_s, written contiguously in scan-time order.

    jobs: (scan_name, dram_src, idx_tile_or_None, is_emb). For is_emb the idx
    tile holds embedding row ids (fp32 gather + cast); otherwise rows of src
    are read contiguously (idx None) or gathered (idx set, layer-2 backward).
    """
    D = KD * 128
    with tc.tile_pool(name="pw", bufs=1) as wpool, \
         tc.tile_pool(name="pg", bufs=3) as gpool, \
         tc.tile_pool(name="pgT", bufs=3) as tpool, \
         tc.tile_pool(name="pps", bufs=4, space="PSUM") as ppool, \
         tc.tile_pool(name="pout", bufs=3) as opool:
        wsb = {}
        for s, _, _, _ in jobs:
            wsb[s] = wpool.tile([128, KD, G], BF16, tag=f"w{s}", name=f"wih_{s}")
            for k in range(KD):
                nc.gpsimd.dma_start(wsb[s][:, k, :], wih[s][128 * k:128 * (k + 1), :])
        for c in range(nchunk):
            for s, dsrc, idx, is_emb in jobs:
                if is_emb:
                    e32 = gpool.tile([128, D], F32, tag="e32")
                    nc.gpsimd.indirect_dma_start(
                        out=e32[:], out_offset=None, in_=dsrc[:],
                        in_offset=IndirectOffsetOnAxis(ap=idx[:, c:c + 1], axis=0))
                    xin = gpool.tile([128, D], BF16, tag="e16")
                    nc.vector.tensor_copy(xin[:], e32[:])
                else:
                    fsrc, fidx, bsrc, bidx = dsrc
                    xin = gpool.tile([128, D], BF16, tag="e16")
                    for src_t, sidx, lo in ((fsrc, fidx, 0), (bsrc, bidx, H)):
                        if sidx is None:
                            nc.gpsimd.dma_start(xin[:, lo:lo + H],
                                                src_t[128 * c:128 * (c + 1), :])
                        else:
                            nc.gpsimd.indirect_dma_start(
                                out=xin[:, lo:lo + H], out_offset=None, in_=src_t[:],
                                in_offset=IndirectOffsetOnAxis(ap=sidx[:, c:c + 1], axis=0))
                xT = tpool.tile([128, KD, 128], BF16, tag="xT")
                for k in range(KD):
                    nc.sync.dma_start_transpose(
                        xT[:, k, :], xin[:, 128 * k:128 * (k + 1)])
                gout = opool.tile([128, G], BF16, tag="gout")
                for n in range(G // 512):
                    ps = ppool.tile([128, 512], F32, name="pps")
                    for k in range(KD):
                        nc.tensor.matmul(
                            ps[:], xT[:, k, :], wsb[s][:, k, 512 * n:512 * (n + 1)],
                            start=(k == 0), stop=(k == KD - 1))
                    nc.vector.tensor_tensor(
                        out=gout[:, 512 * n:512 * (n + 1)], in0=ps[:],
                        in1=bias_sb[s][:, 512 * n:512 * (n + 1)],
                        op=ALU.add)
                nc.gpsimd.dma_start(gx[s][128 * c:128 * (c + 1), :], gout[:])


def _scan_phase(nc, tc, Tn, Bl, TC, RC, KH, scans, whh, gx, hout,
                mask_sb, id_sb):
    """Software-pipelined gx injection: next step's gx lands in PSUM via
    identity matmuls during this step's idle PE window; recurrent matmuls
    then accumulate onto it (start=False) and ScalarE reads gates straight
    from PSUM. Gates live in two 2-bank halves (A: i,f / B: o,g) so slots
    free as soon as their sigmoid/tanh reads finish."""
    gxv = {s: gx[s].ap().rearrange("(b t) d -> b t d", b=Bl) for s in scans}
    houtv = {s: hout[s].ap().rearrange("(b t) d -> b t d", b=Bl) for s in scans}
    H2 = 2 * H
    with tc.tile_pool(name="sw", bufs=1) as wpool, \
         tc.tile_pool(name="sgx", bufs=4) as gxpool, \
         tc.tile_pool(name="sst", bufs=1) as stpool, \
         tc.tile_pool(name="sps", bufs=4, space="PSUM") as pspool, \
         tc.tile_pool(name="swk", bufs=3) as wkpool, \
         tc.tile_pool(name="shT", bufs=3) as htpool, \
         tc.tile_pool(name="srng", bufs=3) as rpool:
        wsb, c_st, hT = {}, {}, {}
        for s in scans:
            wsb[s] = wpool.tile([128, KH, G], BF16, tag=f"whh{s}", name=f"whh_{s}")
            for k in range(KH):
                nc.gpsimd.dma_start(wsb[s][:, k, :], whh[s][128 * k:128 * (k + 1), :])
            c_st[s] = stpool.tile([Bl, H], F32, tag=f"c{s}", name=f"c_{s}")
            nc.vector.memset(c_st[s][:], 0.0)
            hT[s] = htpool.tile([128, KH * Bl], BF16, tag="hT", name="hT0")
            nc.vector.memset(hT[s][:], 0.0)
        gxc = {s: None for s in scans}
        gA = {s: None for s in scans}
        gB = {s: None for s in scans}
        ring = {s: None for s in scans}

        def load_gx(tt):
            for s in scans:
                gxc[s] = gxpool.tile([Bl, TC, G], BF16, tag="gx", name="gxc")
                nc.gpsimd.dma_start(gxc[s][:], gxv[s][:, tt:tt + TC, :])

        def inject(tt, only=None):
            # psum halves for step tt, pre-filled with gx via identity matmuls
            for s in (scans if only is None else [only]):
                gA[s] = pspool.tile([Bl, H2], F32, tag="ps", name="gA")
                gB[s] = pspool.tile([Bl, H2], F32, tag="ps", name="gB")
                for half, lo in ((gA[s], 0), (gB[s], H2)):
                    for n in range(2):
                        nc.tensor.matmul(
                            half[:, 512 * n:512 * (n + 1)], id_sb[:],
                            gxc[s][:, tt % TC, lo + 512 * n:lo + 512 * (n + 1)],
                            start=True, stop=False, skip_group_check=True)

        load_gx(0)
        inject(0)
        for t in range(Tn):
            # recurrent matmuls accumulate onto the injected gx; ScalarE reads
            # gates from PSUM as each half-group completes
            gact = {}
            for s in scans:
                # A half: i (cols 0:512), f (512:1024); B half: o, g
                for half, cols in ((gA[s], (0, 1)), (gB[s], (3, 2))):
                    for n in cols:
                        dst = half[:, 512 * (n % 2):512 * (n % 2 + 1)]
                        for k in range(KH):
                            nc.tensor.matmul(dst,
                                             hT[s][:, Bl * k:Bl * (k + 1)],
                                             wsb[s][:, k, 512 * n:512 * (n + 1)],
                                             start=False, stop=(k == KH - 1),
                                             skip_group_check=True)
            for s in scans:
                gact[s] = wkpool.tile([Bl, G], F32, tag="gact", name="gact")
                if t % RC == 0:
                    ring[s] = rpool.tile([Bl, RC, H], BF16, tag="ring", name="ring")
            for s in scans:
                nc.scalar.activation(gact[s][:, 0:H2], gA[s][:], AF.Sigmoid)
            for s in scans:
                nc.scalar.activation(gact[s][:, 3 * H:G], gB[s][:, H:H2], AF.Tanh)
            for s in scans:
                nc.scalar.activation(gact[s][:, H2:3 * H], gB[s][:, 0:H], AF.Sigmoid)
            t1, t2, tch, h16 = {}, {}, {}, {}
            for s in scans:
                t1[s] = wkpool.tile([Bl, H], F32, tag="t1", name="t1")
                nc.vector.tensor_tensor(out=t1[s][:], in0=gact[s][:, H:H2],
                                        in1=c_st[s][:], op=ALU.mult)
            for s in scans:
                t2[s] = wkpool.tile([Bl, H], F32, tag="t2", name="t2")
                nc.vector.tensor_tensor(out=t2[s][:], in0=gact[s][:, 0:H],
                                        in1=gact[s][:, 3 * H:G], op=ALU.mult)
            for s in scans:
                nc.vector.tensor_tensor(out=c_st[s][:], in0=t1[s][:], in1=t2[s][:],
                                        op=ALU.add)
            for s in scans:
                tch[s] = wkpool.tile([Bl, H], F32, tag="tch", name="tch")
                nc.scalar.activation(tch[s][:], c_st[s][:], AF.Tanh)
            for s in scans:
                h16[s] = wkpool.tile([Bl, H], BF16, tag="h16", name="h16")
                nc.vector.tensor_tensor(out=h16[s][:], in0=gact[s][:, H2:3 * H],
                                        in1=tch[s][:], op=ALU.mult)
            # allocate transpose psum tiles first (keeps the proven slot
            # rotation), then emit next step's gx injects BEFORE the transpose
            # matmuls so they fill the PE window spent waiting for h16
            hT_ps = {}
            for s in scans:
                hT_ps[s] = pspool.tile([128, KH * Bl], F32, tag="ps", name="hT_ps")
            if t + 1 < Tn:
                if (t + 1) % TC == 0:
                    load_gx(t + 1)
                inject(t + 1, only=scans[0])
            for s in scans:
                for k in range(KH):
                    nc.tensor.matmul(hT_ps[s][:, Bl * k:Bl * (k + 1)],
                                     h16[s][:, 128 * k:128 * (k + 1)], id_sb[:],
                                     start=True, stop=True)
                hTn = htpool.tile([128, KH * Bl], BF16, tag="hT", name="hTn")
                nc.scalar.activation(hTn[:], hT_ps[s][:], AF.Copy)
                hT[s] = hTn
            if t + 1 < Tn:
                inject(t + 1, only=scans[1])
            for s in scans:
                nc.vector.tensor_scalar_mul(ring[s][:, t % RC, :], h16[s][:],
                                            mask_sb[:, t:t + 1])
                if (t + 1) % RC == 0:
                    t0r = t + 1 - RC
                    nc.gpsimd.dma_start(houtv[s][:, t0r:t0r + RC, :], ring[s][:])


def _prep_inputs(inputs, Tn=T, Bl=BL, ncores=NC):
    """Host-side sharding + weight preprocessing. Returns per-core in_maps."""
    x = np.asarray(inputs["x"]).astype(np.int32)
    lengths = np.asarray(inputs["lengths"]).astype(np.int32)
    emb = np.asarray(inputs["emb"], dtype=np.float32)
    ntok = Bl * Tn

    com = {"emb": emb, "ident": np.eye(16, dtype=ml_dtypes.bfloat16)}
    for s in ("f1", "b1", "f2", "b2"):
        w_ih = np.asarray(inputs[f"W_ih_{s}"], np.float32)[_GATE_PERM]
        w_hh = np.asarray(inputs[f"W_hh_{s}"], np.float32)[_GATE_PERM]
        b = np.asarray(inputs[f"b_{s}"], np.float32)[_GATE_PERM]
        com[f"wihT_{s}"] = np.ascontiguousarray(w_ih.T).astype(ml_dtypes.bfloat16)
        com[f"whhT_{s}"] = np.ascontiguousarray(w_hh.T).astype(ml_dtypes.bfloat16)
        com[f"bias_{s}"] = np.tile(b.reshape(1, G), (128, 1))
    com["wclsT"] = np.ascontiguousarray(
        np.asarray(inputs["W_cls"], np.float32).T).astype(ml_dtypes.bfloat16)
    com["bcls"] = np.asarray(inputs["b_cls"], np.float32).reshape(TAGS, 1)

    def chunked(a):  # [ntok] -> [128, ntok//128] with chunk c in column c
        return np.ascontiguousarray(a.reshape(-1).reshape(ntok // 128, 128).T)

    in_maps = []
    for c in range(ncores):
        xs = x[Bl * c:Bl * (c + 1), :Tn]
        ls = np.minimum(lengths[Bl * c:Bl * (c + 1)], Tn)
        ts = np.arange(Tn)[None, :]
        rev = np.where(ts < ls[:, None], ls[:, None] - 1 - ts, ts)  # [Bl,Tn]
        xrev = np.take_along_axis(xs, rev, axis=1)
        flat_rev = (np.arange(Bl)[:, None] * Tn + rev).astype(np.int32)
        m = {
            "xf_idx": chunked(xs),
            "xb_idx": chunked(xrev),
            "rev128": chunked(flat_rev),
            "rev16": np.ascontiguousarray(flat_rev),
            "mask": (ts < ls[:, None]).astype(np.float32),
        }
        m.update(com)
        in_maps.append(m)
    return in_maps


_CACHED = {}


def kernel(**inputs) -> np.ndarray:
    if "nc" not in _CACHED:
        nc = bacc.Bacc("TRN2", target_bir_lowering=False, debug=False,
                       num_devices=NC)
        _build(nc)
        nc.compile()
        _CACHED["nc"] = nc
    nc = _CACHED["nc"]
    in_maps = _prep_inputs(inputs)
    res = run_bass_kernel_spmd(nc, in_maps, core_ids=list(range(NC)), trace=False)
    outs = []
    for c in range(NC):
        lt = res.results[c]["logitsT"]  # [TAGS, ntok]
        outs.append(np.ascontiguousarray(lt.T.reshape(BL, T, TAGS)))
    return np.concatenate(outs, axis=0).astype(np.float32)



# revision 2
# speedup vs baseline: 1.8515x; 1.8515x over previous
"""BiLSTM tagger kernel for 8 Trainium2 NeuronCores.

Strategy: data-parallel over batch (16 sequences per core, weights
replicated). Per core, the two directions of each BiLSTM layer run as
col-group-packed scans: scan0 lives entirely at SBUF/PSUM partitions 0:16
(PE column group 0), scan1 at partitions 32:48 (column group 1), so their
recurrent matmuls execute CONCURRENTLY on different 32-column strips of
the 128x128 PE array (tile_position col tiling). All matmuls run in bf16
(fp32 matmul is 4x slower on TRN2); PSUM accumulation stays fp32.

Recurrent step layout: stationary = h^T chunks [128,16], moving = W_hh^T
slices, psum gates live in two [48, 1024] tiles (A: i,f / B: o,g) whose
partition strips 0:16 / 32:48 hold scan0 / scan1 — the strips share PSUM
banks; has_written tracking is per-partition so the per-strip
inject(start=True) + accumulate(start=False) groups are independent
(validated on HW). gx (input projections, precomputed per layer into HBM)
is injected into PSUM via identity matmuls during the previous step's
tail. h is re-transposed each step with four tiny matmuls against a 16x16
identity per scan (row groups 0 / 1, also concurrent). Backward scans
consume inputs pre-reversed per sequence length (host permutation indices
+ indirect DMA); outputs stay in scan order and are un-reversed by the
consumers' row gathers. Steps past a sequence's length compute garbage
that is masked to zero and cannot contaminate earlier steps.
"""

import sys

for _p in ("/opt/trn_rl_repo",):
    if _p not in sys.path:
        sys.path.append(_p)

import numpy as np
import ml_dtypes

import concourse.bass as bass
import concourse.tile as tile
from concourse import bacc, mybir
from concourse.bass import IndirectOffsetOnAxis
from concourse.bass_utils import run_bass_kernel_spmd

F32 = mybir.dt.float32
BF16 = mybir.dt.bfloat16
I32 = mybir.dt.int32
AF = mybir.ActivationFunctionType
ALU = mybir.AluOpType

# problem sizes (full / per-core)
B, T, V, E, H, TAGS = 128, 512, 50000, 256, 512, 64
NC = 8
BL = B // NC   # 16 sequences per core
G = 4 * H      # 2048 gate width

# permutation taking pytorch gate order i,f,g,o -> i,f,o,g (sigmoid block first)
_GATE_PERM = np.concatenate([
    np.arange(0, H), np.arange(H, 2 * H), np.arange(3 * H, 4 * H),
    np.arange(2 * H, 3 * H)])

# partition strip (= PE column group offset) per scan slot
_STRIP = (0, 32)


def _build(nc, Tn=T, Bl=BL, TC=2, RC=4):
    """Emit the per-core program. Tn shrinkable for dev testing."""
    ntok = Bl * Tn
    nchunk = ntok // 128
    KE = E // 128       # k-chunks for layer-1 input proj
    KH2 = 2 * H // 128  # k-chunks for layer-2 input proj / classifier
    KH = H // 128       # k-chunks for recurrent
    assert ntok % 128 == 0

    # ---- dram I/O ----
    emb = nc.dram_tensor("emb", [V, E], F32, kind="ExternalInput")
    xf_idx = nc.dram_tensor("xf_idx", [128, nchunk], I32, kind="ExternalInput")
    xb_idx = nc.dram_tensor("xb_idx", [128, nchunk], I32, kind="ExternalInput")
    rev128 = nc.dram_tensor("rev128", [128, nchunk], I32, kind="ExternalInput")
    mask = nc.dram_tensor("mask", [Bl, Tn], F32, kind="ExternalInput")
    ident = nc.dram_tensor("ident", [16, 16], BF16, kind="ExternalInput")

    wih, whh, biasd = {}, {}, {}
    for s, din in (("f1", E), ("b1", E), ("f2", 2 * H), ("b2", 2 * H)):
        wih[s] = nc.dram_tensor(f"wihT_{s}", [din, G], BF16, kind="ExternalInput")
        whh[s] = nc.dram_tensor(f"whhT_{s}", [H, G], BF16, kind="ExternalInput")
        biasd[s] = nc.dram_tensor(f"bias_{s}", [128, G], F32, kind="ExternalInput")
    wcls = nc.dram_tensor("wclsT", [2 * H, TAGS], BF16, kind="ExternalInput")
    bcls = nc.dram_tensor("bcls", [TAGS, 1], F32, kind="ExternalInput")

    gx = {s: nc.dram_tensor(f"gx_{s}", [ntok, G], BF16)
          for s in ("f1", "b1", "f2", "b2")}
    # per-direction layer outputs; backward halves stay in scan order and are
    # un-reversed by the consumers' row gathers (no per-step scatters)
    hout = {s: nc.dram_tensor(f"hout_{s}", [ntok, H], BF16)
            for s in ("f1", "b1", "f2", "b2")}
    logitsT = nc.dram_tensor("logitsT", [TAGS, ntok], F32, kind="ExternalOutput")

    with tile.TileContext(nc) as tc:
        with tc.tile_pool(name="const", bufs=1) as cpool:
            def load_const(nm, shape, dt, src_ap):
                t = cpool.tile(shape, dt, name=nm, tag=nm)
                nc.gpsimd.dma_start(t[:], src_ap)
                return t

            xf_sb = load_const("xf_sb", [128, nchunk], I32, xf_idx[:])
            xb_sb = load_const("xb_sb", [128, nchunk], I32, xb_idx[:])
            rev128_sb = load_const("rev128_sb", [128, nchunk], I32, rev128[:])
            bcls_sb = load_const("bcls_sb", [TAGS, 1], F32, bcls[:])
            bias_sb = {s: load_const(f"bias_sb_{s}", [128, G], F32, biasd[s][:])
                       for s in ("f1", "b1", "f2", "b2")}
            # identity + mask replicated into both scan strips
            id_all = cpool.tile([48, 16], BF16, name="id_all")
            mask_all = cpool.tile([48, Tn], F32, name="mask_all")
            for p in _STRIP:
                nc.gpsimd.dma_start(id_all[p:p + 16, :], ident[:])
                nc.gpsimd.dma_start(mask_all[p:p + 16, :], mask[:])
            wcls_sb = cpool.tile([128, KH2, TAGS], BF16, name="wcls_sb")
            for k in range(KH2):
                nc.gpsimd.dma_start(wcls_sb[:, k, :], wcls[128 * k:128 * (k + 1), :])

            # layer-1 input projections (inputs gathered from embedding table)
            _proj_phase(nc, tc, nchunk, KE, wih=wih, bias_sb=bias_sb, gx=gx,
                        jobs=[("f1", emb, xf_sb, True), ("b1", emb, xb_sb, True)])
            # layer-1 scans
            _scan_phase(nc, tc, Tn, Bl, TC, RC, KH,
                        scans=("f1", "b1"), whh=whh, gx=gx, hout=hout,
                        mask_all=mask_all, id_all=id_all)
            # layer-2 input projections: input token (b,t) for the fwd scan is
            # [f1h[t], s1h[rev(t)]]; for the bwd scan it is [f1h[rev(t)], s1h[t]]
            _proj_phase(nc, tc, nchunk, KH2, wih=wih, bias_sb=bias_sb, gx=gx,
                        jobs=[("f2", (hout["f1"], None, hout["b1"], rev128_sb), None, False),
                              ("b2", (hout["f1"], rev128_sb, hout["b1"], None), None, False)])
            # layer-2 scans
            _scan_phase(nc, tc, Tn, Bl, TC, RC, KH,
                        scans=("f2", "b2"), whh=whh, gx=gx, hout=hout,
                        mask_all=mask_all, id_all=id_all)

            # classifier: logits^T = W_cls @ out2^T + b_cls
            with tc.tile_pool(name="cls", bufs=3) as gp, \
                 tc.tile_pool(name="clsT", bufs=3) as gtp, \
                 tc.tile_pool(name="clsps", bufs=4, space="PSUM") as pp, \
                 tc.tile_pool(name="clso", bufs=3) as op:
                for c in range(nchunk):
                    o2 = gp.tile([128, 2 * H], BF16, tag="in")
                    nc.gpsimd.dma_start(o2[:, 0:H], hout["f2"][128 * c:128 * (c + 1), :])
                    nc.gpsimd.indirect_dma_start(
                        out=o2[:, H:2 * H], out_offset=None, in_=hout["b2"][:],
                        in_offset=IndirectOffsetOnAxis(ap=rev128_sb[:, c:c + 1], axis=0))
                    o2T = gtp.tile([128, KH2, 128], BF16, tag="inT")
                    for k in range(KH2):
                        nc.sync.dma_start_transpose(
                            o2T[:, k, :], o2[:, 128 * k:128 * (k + 1)])
                    ps = pp.tile([TAGS, 128], F32, name="clsps_t")
                    for k in range(KH2):
                        nc.tensor.matmul(ps[:], wcls_sb[:, k, :], o2T[:, k, :],
                                         start=(k == 0), stop=(k == KH2 - 1))
                    lg = op.tile([TAGS, 128], F32, tag="lg")
                    nc.scalar.activation(lg[:], ps[:], AF.Identity,
                                         bias=bcls_sb[:, 0:1])
                    nc.gpsimd.dma_start(logitsT[:, 128 * c:128 * (c + 1)], lg[:])
    return nc


def _proj_phase(nc, tc, nchunk, KD, wih, bias_sb, gx, jobs):
    """gx_s = input @ W_ih_s^T + b_s, written contiguously in scan-time order.

    jobs: (scan_name, dram_src, idx_tile_or_None, is_emb). For is_emb the idx
    tile holds embedding row ids (fp32 gather + cast); otherwise rows of src
    are read contiguously (idx None) or gathered (idx set, layer-2 backward).
    """
    D = KD * 128
    with tc.tile_pool(name="pw", bufs=1) as wpool, \
         tc.tile_pool(name="pg", bufs=3) as gpool, \
         tc.tile_pool(name="pgT", bufs=3) as tpool, \
         tc.tile_pool(name="pps", bufs=4, space="PSUM") as ppool, \
         tc.tile_pool(name="pout", bufs=3) as opool:
        wsb = {}
        for s, _, _, _ in jobs:
            wsb[s] = wpool.tile([128, KD, G], BF16, tag=f"w{s}", name=f"wih_{s}")
            for k in range(KD):
                nc.gpsimd.dma_start(wsb[s][:, k, :], wih[s][128 * k:128 * (k + 1), :])
        for c in range(nchunk):
            for s, dsrc, idx, is_emb in jobs:
                if is_emb:
                    e32 = gpool.tile([128, D], F32, tag="e32")
                    nc.gpsimd.indirect_dma_start(
                        out=e32[:], out_offset=None, in_=dsrc[:],
                        in_offset=IndirectOffsetOnAxis(ap=idx[:, c:c + 1], axis=0))
                    xin = gpool.tile([128, D], BF16, tag="e16")
                    nc.vector.tensor_copy(xin[:], e32[:])
                else:
                    fsrc, fidx, bsrc, bidx = dsrc
                    xin = gpool.tile([128, D], BF16, tag="e16")
                    for src_t, sidx, lo in ((fsrc, fidx, 0), (bsrc, bidx, H)):
                        if sidx is None:
                            nc.gpsimd.dma_start(xin[:, lo:lo + H],
                                                src_t[128 * c:128 * (c + 1), :])
                        else:
                            nc.gpsimd.indirect_dma_start(
                                out=xin[:, lo:lo + H], out_offset=None, in_=src_t[:],
                                in_offset=IndirectOffsetOnAxis(ap=sidx[:, c:c + 1], axis=0))
                xT = tpool.tile([128, KD, 128], BF16, tag="xT")
                for k in range(KD):
                    nc.sync.dma_start_transpose(
                        xT[:, k, :], xin[:, 128 * k:128 * (k + 1)])
                gout = opool.tile([128, G], BF16, tag="gout")
                for n in range(G // 512):
                    ps = ppool.tile([128, 512], F32, name="pps")
                    for k in range(KD):
                        nc.tensor.matmul(
                            ps[:], xT[:, k, :], wsb[s][:, k, 512 * n:512 * (n + 1)],
                            start=(k == 0), stop=(k == KD - 1))
                    nc.vector.tensor_tensor(
                        out=gout[:, 512 * n:512 * (n + 1)], in0=ps[:],
                        in1=bias_sb[s][:, 512 * n:512 * (n + 1)],
                        op=ALU.add)
                nc.gpsimd.dma_start(gx[s][128 * c:128 * (c + 1), :], gout[:])


def _scan_phase(nc, tc, Tn, Bl, TC, RC, KH, scans, whh, gx, hout,
                mask_all, id_all):
    """Col-group packed scans: scan i occupies partition strip _STRIP[i]
    (PE column group i). Software-pipelined gx injection: next step's gx
    lands in PSUM via identity matmuls during this step's idle PE window;
    recurrent matmuls then accumulate onto it (start=False) and ScalarE
    reads gates straight from PSUM. Gates live in two 2-bank [48, 1024]
    tiles (A: i,f / B: o,g) whose partition strips are per-scan."""
    gxv = {s: gx[s].ap().rearrange("(b t) d -> b t d", b=Bl) for s in scans}
    houtv = {s: hout[s].ap().rearrange("(b t) d -> b t d", b=Bl) for s in scans}
    H2 = 2 * H
    strip = {s: _STRIP[i] for i, s in enumerate(scans)}

    def sl(s):
        return slice(strip[s], strip[s] + 16)

    with tc.tile_pool(name="sw", bufs=1) as wpool, \
         tc.tile_pool(name="sgx", bufs=4) as gxpool, \
         tc.tile_pool(name="sst", bufs=1) as stpool, \
         tc.tile_pool(name="sps", bufs=4, space="PSUM") as pspool, \
         tc.tile_pool(name="swk", bufs=3) as wkpool, \
         tc.tile_pool(name="shT", bufs=3) as htpool, \
         tc.tile_pool(name="srng", bufs=3) as rpool:
        wsb, hT = {}, {}
        for s in scans:
            wsb[s] = wpool.tile([128, KH, G], BF16, tag=f"whh{s}", name=f"whh_{s}")
            for k in range(KH):
                nc.gpsimd.dma_start(wsb[s][:, k, :], whh[s][128 * k:128 * (k + 1), :])
            hT[s] = htpool.tile([128, KH * Bl], BF16, tag="hT", name="hT0")
            nc.vector.memset(hT[s][:], 0.0)
        c_all = stpool.tile([48, H], F32, tag="c", name="c_all")
        nc.vector.memset(c_all[:], 0.0)
        gxc = [None]
        gA = [None]
        gB = [None]
        ring = [None]

        def load_gx(tt):
            gxc[0] = gxpool.tile([48, TC, G], BF16, tag="gx", name="gxc")
            for s in scans:
                nc.gpsimd.dma_start(gxc[0][sl(s), :, :], gxv[s][:, tt:tt + TC, :])

        def inject(tt):
            # psum halves for step tt, pre-filled with gx via identity
            # matmuls; the two scans' strips pack onto col groups 0/1
            gA[0] = pspool.tile([48, H2], F32, tag="ps", name="gA")
            gB[0] = pspool.tile([48, H2], F32, tag="ps", name="gB")
            for half, lo in ((gA[0], 0), (gB[0], H2)):
                for n in range(2):
                    for s in scans:
                        p = strip[s]
                        nc.tensor.matmul(
                            half[sl(s), 512 * n:512 * (n + 1)], id_all[sl(s), :],
                            gxc[0][sl(s), tt % TC, lo + 512 * n:lo + 512 * (n + 1)],
                            start=True, stop=False, skip_group_check=True,
                            tile_position=(p, p))

        load_gx(0)
        inject(0)
        for t in range(Tn):
            # recurrent matmuls accumulate onto the injected gx, the two
            # scans' chains interleaved pairwise so they run concurrently on
            # col groups 0/1; ScalarE reads gates from PSUM per half-group
            gAc, gBc = gA[0], gB[0]
            for half, cols in ((gAc, (0, 1)), (gBc, (3, 2))):
                for k in range(KH):
                    for n in cols:
                        dst_lo = 512 * (n % 2)
                        for s in scans:
                            nc.tensor.matmul(
                                half[sl(s), dst_lo:dst_lo + 512],
                                hT[s][:, Bl * k:Bl * (k + 1)],
                                wsb[s][:, k, 512 * n:512 * (n + 1)],
                                start=False, stop=(k == KH - 1),
                                skip_group_check=True,
                                tile_position=(0, strip[s]))
            gact = wkpool.tile([48, G], F32, tag="gact", name="gact")
            if t % RC == 0:
                ring[0] = rpool.tile([48, RC, H], BF16, tag="ring", name="ring")
            for s in scans:
                nc.scalar.activation(gact[sl(s), 0:H2], gAc[sl(s), :], AF.Sigmoid)
            for s in scans:
                nc.scalar.activation(gact[sl(s), 3 * H:G], gBc[sl(s), H:H2], AF.Tanh)
            for s in scans:
                nc.scalar.activation(gact[sl(s), H2:3 * H], gBc[sl(s), 0:H], AF.Sigmoid)
            t1 = wkpool.tile([48, H], F32, tag="t1", name="t1")
            t2 = wkpool.tile([48, H], F32, tag="t2", name="t2")
            for s in scans:
                nc.vector.tensor_tensor(out=t1[sl(s), :], in0=gact[sl(s), H:H2],
                                        in1=c_all[sl(s), :], op=ALU.mult)
            for s in scans:
                nc.vector.tensor_tensor(out=t2[sl(s), :], in0=gact[sl(s), 0:H],
                                        in1=gact[sl(s), 3 * H:G], op=ALU.mult)
            for s in scans:
                nc.vector.tensor_tensor(out=c_all[sl(s), :], in0=t1[sl(s), :],
                                        in1=t2[sl(s), :], op=ALU.add)
            tch = wkpool.tile([48, H], F32, tag="tch", name="tch")
            for s in scans:
                nc.scalar.activation(tch[sl(s), :], c_all[sl(s), :], AF.Tanh)
            h16 = wkpool.tile([48, H], BF16, tag="h16", name="h16")
            for s in scans:
                nc.vector.tensor_tensor(out=h16[sl(s), :], in0=gact[sl(s), H2:3 * H],
                                        in1=tch[sl(s), :], op=ALU.mult)
            # allocate transpose psum tiles first (keeps the slot rotation),
            # then emit next step's gx injects BEFORE the transpose matmuls
            # so they fill the PE window spent waiting for h16
            hT_ps = {}
            for s in scans:
                hT_ps[s] = pspool.tile([128, KH * Bl], F32, tag="ps", name="hT_ps")
            if t + 1 < Tn:
                if (t + 1) % TC == 0:
                    load_gx(t + 1)
                inject(t + 1)
            for s in scans:
                for k in range(KH):
                    nc.tensor.matmul(hT_ps[s][:, Bl * k:Bl * (k + 1)],
                                     h16[sl(s), 128 * k:128 * (k + 1)],
                                     id_all[sl(s), :], start=True, stop=True,
                                     tile_position=(strip[s], 0))
                hTn = htpool.tile([128, KH * Bl], BF16, tag="hT", name="hTn")
                nc.scalar.activation(hTn[:], hT_ps[s][:], AF.Copy)
                hT[s] = hTn
            for s in scans:
                nc.vector.tensor_scalar_mul(ring[0][sl(s), t % RC, :], h16[sl(s), :],
                                            mask_all[sl(s), t:t + 1])
            if (t + 1) % RC == 0:
                t0r = t + 1 - RC
                for s in scans:
                    nc.gpsimd.dma_start(houtv[s][:, t0r:t0r + RC, :],
                                        ring[0][sl(s), :, :])


def _prep_inputs(inputs, Tn=T, Bl=BL, ncores=NC):
    """Host-side sharding + weight preprocessing. Returns per-core in_maps."""
    x = np.asarray(inputs["x"]).astype(np.int32)
    lengths = np.asarray(inputs["lengths"]).astype(np.int32)
    emb = np.asarray(inputs["emb"], dtype=np.float32)
    ntok = Bl * Tn

    com = {"emb": emb, "ident": np.eye(16, dtype=ml_dtypes.bfloat16)}
    for s in ("f1", "b1", "f2", "b2"):
        w_ih = np.asarray(inputs[f"W_ih_{s}"], np.float32)[_GATE_PERM]
        w_hh = np.asarray(inputs[f"W_hh_{s}"], np.float32)[_GATE_PERM]
        b = np.asarray(inputs[f"b_{s}"], np.float32)[_GATE_PERM]
        com[f"wihT_{s}"] = np.ascontiguousarray(w_ih.T).astype(ml_dtypes.bfloat16)
        com[f"whhT_{s}"] = np.ascontiguousarray(w_hh.T).astype(ml_dtypes.bfloat16)
        com[f"bias_{s}"] = np.tile(b.reshape(1, G), (128, 1))
    com["wclsT"] = np.ascontiguousarray(
        np.asarray(inputs["W_cls"], np.float32).T).astype(ml_dtypes.bfloat16)
    com["bcls"] = np.asarray(inputs["b_cls"], np.float32).reshape(TAGS, 1)

    def chunked(a):  # [ntok] -> [128, ntok//128] with chunk c in column c
        return np.ascontiguousarray(a.reshape(-1).reshape(ntok // 128, 128).T)

    in_maps = []
    for c in range(ncores):
        xs = x[Bl * c:Bl * (c + 1), :Tn]
        ls = np.minimum(lengths[Bl * c:Bl * (c + 1)], Tn)
        ts = np.arange(Tn)[None, :]
        rev = np.where(ts < ls[:, None], ls[:, None] - 1 - ts, ts)  # [Bl,Tn]
        xrev = np.take_along_axis(xs, rev, axis=1)
        flat_rev = (np.arange(Bl)[:, None] * Tn + rev).astype(np.int32)
        m = {
            "xf_idx": chunked(xs),
            "xb_idx": chunked(xrev),
            "rev128": chunked(flat_rev),
            "mask": (ts < ls[:, None]).astype(np.float32),
        }
        m.update(com)
        in_maps.append(m)
    return in_maps


_CACHED = {}


def kernel(**inputs) -> np.ndarray:
    if "nc" not in _CACHED:
        nc = bacc.Bacc("TRN2", target_bir_lowering=False, debug=False,
                       num_devices=NC)
        _build(nc)
        nc.compile()
        _CACHED["nc"] = nc
    nc = _CACHED["nc"]
    in_maps = _prep_inputs(inputs)
    res = run_bass_kernel_spmd(nc, in_maps, core_ids=list(range(NC)), trace=False)
    outs = []
    for c in range(NC):
        lt = res.results[c]["logitsT"]  # [TAGS, ntok]
        outs.append(np.ascontiguousarray(lt.T.reshape(BL, T, TAGS)))
    return np.concatenate(outs, axis=0).astype(np.float32)


# revision 6
# speedup vs baseline: 1.8658x; 1.0077x over previous
"""BiLSTM tagger kernel for 8 Trainium2 NeuronCores.

Strategy: data-parallel over batch (16 sequences per core, weights
replicated). Per core, the two directions of each BiLSTM layer run as
col-group-packed scans: scan0 lives entirely at SBUF/PSUM partitions 0:16
(PE column group 0), scan1 at partitions 32:48 (column group 1), so their
recurrent matmuls execute CONCURRENTLY on different 32-column strips of
the 128x128 PE array (tile_position col tiling). All matmuls run in bf16
(fp32 matmul is 4x slower on TRN2); PSUM accumulation stays fp32.

Recurrent step layout: stationary = h^T chunks [128,16], moving = W_hh^T
slices, psum gates live in two [48, 1024] tiles (A: i,f / B: o,g) whose
partition strips 0:16 / 32:48 hold scan0 / scan1 — the strips share PSUM
banks; has_written tracking is per-partition so the per-strip
inject(start=True) + accumulate(start=False) groups are independent
(validated on HW). gx (input projections, precomputed per layer into HBM)
is injected into PSUM via identity matmuls during the previous step's
tail. h is re-transposed each step with four tiny matmuls against a 16x16
identity per scan (row groups 0 / 1, also concurrent). Backward scans
consume inputs pre-reversed per sequence length (host permutation indices
+ indirect DMA); outputs stay in scan order and are un-reversed by the
consumers' row gathers. Steps past a sequence's length compute garbage
that is masked to zero and cannot contaminate earlier steps.
"""

import sys

for _p in ("/opt/trn_rl_repo",):
    if _p not in sys.path:
        sys.path.append(_p)

import numpy as np
import ml_dtypes

import concourse.bass as bass
import concourse.tile as tile
from concourse import bacc, mybir
from concourse.bass import IndirectOffsetOnAxis
from concourse.bass_utils import run_bass_kernel_spmd

F32 = mybir.dt.float32
BF16 = mybir.dt.bfloat16
I32 = mybir.dt.int32
AF = mybir.ActivationFunctionType
ALU = mybir.AluOpType

# problem sizes (full / per-core)
B, T, V, E, H, TAGS = 128, 512, 50000, 256, 512, 64
NC = 8
BL = B // NC   # 16 sequences per core
G = 4 * H      # 2048 gate width

# permutation taking pytorch gate order i,f,g,o -> i,f,o,g (sigmoid block first)
_GATE_PERM = np.concatenate([
    np.arange(0, H), np.arange(H, 2 * H), np.arange(3 * H, 4 * H),
    np.arange(2 * H, 3 * H)])

# partition strip (= PE column group offset) per scan slot
_STRIP = (0, 32)


def _build(nc, Tn=T, Bl=BL, TC=2, RC=4, ablate=()):
    """Emit the per-core program. Tn shrinkable for dev testing.
    ablate: subset of {"scan","proj","cls"} to skip (timing attribution)."""
    ntok = Bl * Tn
    nchunk = ntok // 128
    KE = E // 128       # k-chunks for layer-1 input proj
    KH2 = 2 * H // 128  # k-chunks for layer-2 input proj / classifier
    KH = H // 128       # k-chunks for recurrent
    assert ntok % 128 == 0

    # ---- dram I/O ----
    emb = nc.dram_tensor("emb", [V, E], F32, kind="ExternalInput")
    xf_idx = nc.dram_tensor("xf_idx", [128, nchunk], I32, kind="ExternalInput")
    xb_idx = nc.dram_tensor("xb_idx", [128, nchunk], I32, kind="ExternalInput")
    rev128 = nc.dram_tensor("rev128", [128, nchunk], I32, kind="ExternalInput")
    mask = nc.dram_tensor("mask", [Bl, Tn], F32, kind="ExternalInput")
    ident = nc.dram_tensor("ident", [16, 16], BF16, kind="ExternalInput")

    wih, whh, biasd = {}, {}, {}
    for s, din in (("f1", E), ("b1", E), ("f2", 2 * H), ("b2", 2 * H)):
        wih[s] = nc.dram_tensor(f"wihT_{s}", [din, G], BF16, kind="ExternalInput")
        whh[s] = nc.dram_tensor(f"whhT_{s}", [H, G], BF16, kind="ExternalInput")
        biasd[s] = nc.dram_tensor(f"bias_{s}", [128, G], F32, kind="ExternalInput")
    wcls = nc.dram_tensor("wclsT", [2 * H, TAGS], BF16, kind="ExternalInput")
    bcls = nc.dram_tensor("bcls", [TAGS, 1], F32, kind="ExternalInput")

    gx = {s: nc.dram_tensor(f"gx_{s}", [ntok, G], BF16)
          for s in ("f1", "b1", "f2", "b2")}
    # per-direction layer outputs; backward halves stay in scan order and are
    # un-reversed by the consumers' row gathers (no per-step scatters)
    hout = {s: nc.dram_tensor(f"hout_{s}", [ntok, H], BF16)
            for s in ("f1", "b1", "f2", "b2")}
    logitsT = nc.dram_tensor("logitsT", [TAGS, ntok], F32, kind="ExternalOutput")

    with tile.TileContext(nc) as tc:
        with tc.tile_pool(name="const", bufs=1) as cpool:
            def load_const(nm, shape, dt, src_ap):
                t = cpool.tile(shape, dt, name=nm, tag=nm)
                nc.gpsimd.dma_start(t[:], src_ap)
                return t

            xf_sb = load_const("xf_sb", [128, nchunk], I32, xf_idx[:])
            xb_sb = load_const("xb_sb", [128, nchunk], I32, xb_idx[:])
            rev128_sb = load_const("rev128_sb", [128, nchunk], I32, rev128[:])
            bcls_sb = load_const("bcls_sb", [TAGS, 1], F32, bcls[:])
            bias_sb = {s: load_const(f"bias_sb_{s}", [128, G], F32, biasd[s][:])
                       for s in ("f1", "b1", "f2", "b2")}
            # identity + mask replicated into both scan strips
            id_all = cpool.tile([48, 16], BF16, name="id_all")
            mask_all = cpool.tile([48, Tn], F32, name="mask_all")
            for p in _STRIP:
                nc.gpsimd.dma_start(id_all[p:p + 16, :], ident[:])
                nc.gpsimd.dma_start(mask_all[p:p + 16, :], mask[:])
            wcls_sb = cpool.tile([128, KH2, TAGS], BF16, name="wcls_sb")
            for k in range(KH2):
                nc.gpsimd.dma_start(wcls_sb[:, k, :], wcls[128 * k:128 * (k + 1), :])

            # layer-1 input projections (inputs gathered from embedding table)
            if "proj" not in ablate:
                _proj_phase(nc, tc, nchunk, KE, wih=wih, bias_sb=bias_sb, gx=gx,
                            jobs=[("f1", emb, xf_sb, True), ("b1", emb, xb_sb, True)])
            # layer-1 scans
            if "scan" not in ablate:
                _scan_phase(nc, tc, Tn, Bl, TC, RC, KH,
                            scans=("f1", "b1"), whh=whh, gx=gx, hout=hout,
                            mask_all=mask_all, id_all=id_all)
            # layer-2 input projections: input token (b,t) for the fwd scan is
            # [f1h[t], s1h[rev(t)]]; for the bwd scan it is [f1h[rev(t)], s1h[t]]
            if "proj" not in ablate:
                _proj_phase(nc, tc, nchunk, KH2, wih=wih, bias_sb=bias_sb, gx=gx,
                            jobs=[("f2", (hout["f1"], None, hout["b1"], rev128_sb), None, False),
                                  ("b2", (hout["f1"], rev128_sb, hout["b1"], None), None, False)])
            # layer-2 scans
            if "scan" not in ablate:
                _scan_phase(nc, tc, Tn, Bl, TC, RC, KH,
                            scans=("f2", "b2"), whh=whh, gx=gx, hout=hout,
                            mask_all=mask_all, id_all=id_all)

            if "cls" in ablate:
                lg0 = cpool.tile([TAGS, 128], F32, name="lg0")
                nc.vector.memset(lg0[:], 0.0)
                for c in range(nchunk):
                    nc.gpsimd.dma_start(logitsT[:, 128 * c:128 * (c + 1)], lg0[:])
                return nc
            # classifier: logits^T = W_cls @ out2^T + b_cls
            with tc.tile_pool(name="cls", bufs=3) as gp, \
                 tc.tile_pool(name="clsT", bufs=3) as gtp, \
                 tc.tile_pool(name="clsps", bufs=4, space="PSUM") as pp, \
                 tc.tile_pool(name="clso", bufs=3) as op:
                for c in range(nchunk):
                    o2 = gp.tile([128, 2 * H], BF16, tag="in")
                    nc.gpsimd.dma_start(o2[:, 0:H], hout["f2"][128 * c:128 * (c + 1), :])
                    nc.gpsimd.indirect_dma_start(
                        out=o2[:, H:2 * H], out_offset=None, in_=hout["b2"][:],
                        in_offset=IndirectOffsetOnAxis(ap=rev128_sb[:, c:c + 1], axis=0))
                    o2T = gtp.tile([128, KH2, 128], BF16, tag="inT")
                    for k in range(KH2):
                        nc.sync.dma_start_transpose(
                            o2T[:, k, :], o2[:, 128 * k:128 * (k + 1)])
                    ps = pp.tile([TAGS, 128], F32, name="clsps_t")
                    for k in range(KH2):
                        nc.tensor.matmul(ps[:], wcls_sb[:, k, :], o2T[:, k, :],
                                         start=(k == 0), stop=(k == KH2 - 1))
                    lg = op.tile([TAGS, 128], F32, tag="lg")
                    nc.scalar.activation(lg[:], ps[:], AF.Identity,
                                         bias=bcls_sb[:, 0:1])
                    nc.gpsimd.dma_start(logitsT[:, 128 * c:128 * (c + 1)], lg[:])
    return nc


def _proj_phase(nc, tc, nchunk, KD, wih, bias_sb, gx, jobs):
    """gx_s = input @ W_ih_s^T + b_s, written contiguously in scan-time order.

    jobs: (scan_name, dram_src, idx_tile_or_None, is_emb). For is_emb the idx
    tile holds embedding row ids (fp32 gather + cast); otherwise rows of src
    are read contiguously (idx None) or gathered (idx set, layer-2 backward).
    """
    D = KD * 128
    with tc.tile_pool(name="pw", bufs=1) as wpool, \
         tc.tile_pool(name="pg", bufs=3) as gpool, \
         tc.tile_pool(name="pgT", bufs=3) as tpool, \
         tc.tile_pool(name="pps", bufs=4, space="PSUM") as ppool, \
         tc.tile_pool(name="pout", bufs=3) as opool:
        wsb = {}
        for s, _, _, _ in jobs:
            wsb[s] = wpool.tile([128, KD, G], BF16, tag=f"w{s}", name=f"wih_{s}")
            for k in range(KD):
                nc.gpsimd.dma_start(wsb[s][:, k, :], wih[s][128 * k:128 * (k + 1), :])
        for c in range(nchunk):
            for s, dsrc, idx, is_emb in jobs:
                if is_emb:
                    e32 = gpool.tile([128, D], F32, tag="e32")
                    nc.gpsimd.indirect_dma_start(
                        out=e32[:], out_offset=None, in_=dsrc[:],
                        in_offset=IndirectOffsetOnAxis(ap=idx[:, c:c + 1], axis=0))
                    xin = gpool.tile([128, D], BF16, tag="e16")
                    nc.vector.tensor_copy(xin[:], e32[:])
                else:
                    fsrc, fidx, bsrc, bidx = dsrc
                    xin = gpool.tile([128, D], BF16, tag="e16")
                    for src_t, sidx, lo in ((fsrc, fidx, 0), (bsrc, bidx, H)):
                        if sidx is None:
                            nc.gpsimd.dma_start(xin[:, lo:lo + H],
                                                src_t[128 * c:128 * (c + 1), :])
                        else:
                            nc.gpsimd.indirect_dma_start(
                                out=xin[:, lo:lo + H], out_offset=None, in_=src_t[:],
                                in_offset=IndirectOffsetOnAxis(ap=sidx[:, c:c + 1], axis=0))
                xT = tpool.tile([128, KD, 128], BF16, tag="xT")
                for k in range(KD):
                    nc.sync.dma_start_transpose(
                        xT[:, k, :], xin[:, 128 * k:128 * (k + 1)])
                gout = opool.tile([128, G], BF16, tag="gout")
                for n in range(G // 512):
                    ps = ppool.tile([128, 512], F32, name="pps")
                    for k in range(KD):
                        nc.tensor.matmul(
                            ps[:], xT[:, k, :], wsb[s][:, k, 512 * n:512 * (n + 1)],
                            start=(k == 0), stop=(k == KD - 1))
                    nc.vector.tensor_tensor(
                        out=gout[:, 512 * n:512 * (n + 1)], in0=ps[:],
                        in1=bias_sb[s][:, 512 * n:512 * (n + 1)],
                        op=ALU.add)
                nc.gpsimd.dma_start(gx[s][128 * c:128 * (c + 1), :], gout[:])


def _scan_phase(nc, tc, Tn, Bl, TC, RC, KH, scans, whh, gx, hout,
                mask_all, id_all):
    """Col-group packed scans: scan i occupies partition strip _STRIP[i]
    (PE column group i). Software-pipelined gx injection: next step's gx
    lands in PSUM via identity matmuls during this step's idle PE window;
    recurrent matmuls then accumulate onto it (start=False) and ScalarE
    reads gates straight from PSUM. Gates live in two 2-bank [48, 1024]
    tiles (A: i,f / B: o,g) whose partition strips are per-scan."""
    gxv = {s: gx[s].ap().rearrange("(b t) d -> b t d", b=Bl) for s in scans}
    houtv = {s: hout[s].ap().rearrange("(b t) d -> b t d", b=Bl) for s in scans}
    H2 = 2 * H
    strip = {s: _STRIP[i] for i, s in enumerate(scans)}

    def sl(s):
        return slice(strip[s], strip[s] + 16)

    with tc.tile_pool(name="sw", bufs=1) as wpool, \
         tc.tile_pool(name="sgx", bufs=4) as gxpool, \
         tc.tile_pool(name="sst", bufs=1) as stpool, \
         tc.tile_pool(name="sps", bufs=4, space="PSUM") as pspool, \
         tc.tile_pool(name="swk", bufs=3) as wkpool, \
         tc.tile_pool(name="shT", bufs=3) as htpool, \
         tc.tile_pool(name="srng", bufs=3) as rpool:
        wsb, hT = {}, {}
        for s in scans:
            wsb[s] = wpool.tile([128, KH, G], BF16, tag=f"whh{s}", name=f"whh_{s}")
            for k in range(KH):
                nc.gpsimd.dma_start(wsb[s][:, k, :], whh[s][128 * k:128 * (k + 1), :])
            hT[s] = htpool.tile([128, KH * Bl], BF16, tag="hT", name="hT0")
            nc.vector.memset(hT[s][:], 0.0)
        c_all = stpool.tile([48, H], F32, tag="c", name="c_all")
        nc.vector.memset(c_all[:], 0.0)
        gxc = {}
        gA = [None]
        gB = [None]
        ring = [None]
        nwin = (Tn + TC - 1) // TC

        def load_gx(w):
            # prefetch gx window w (steps w*TC .. w*TC+TC-1)
            tl = gxpool.tile([48, TC, G], BF16, tag="gx", name="gxc")
            for s in scans:
                nc.gpsimd.dma_start(tl[sl(s), :, :],
                                    gxv[s][:, w * TC:(w + 1) * TC, :])
            gxc[w] = tl
            gxc.pop(w - 3, None)

        def inject(tt):
            # psum halves for step tt, pre-filled with gx via identity
            # matmuls; the two scans' strips pack onto col groups 0/1
            gA[0] = pspool.tile([48, H2], F32, tag="ps", name="gA")
            gB[0] = pspool.tile([48, H2], F32, tag="ps", name="gB")
            gxt = gxc[tt // TC]
            for half, lo in ((gA[0], 0), (gB[0], H2)):
                for n in range(2):
                    for s in scans:
                        p = strip[s]
                        nc.tensor.matmul(
                            half[sl(s), 512 * n:512 * (n + 1)], id_all[sl(s), :],
                            gxt[sl(s), tt % TC, lo + 512 * n:lo + 512 * (n + 1)],
                            start=True, stop=False, skip_group_check=True,
                            tile_position=(p, p))

        load_gx(0)
        if nwin > 1:
            load_gx(1)
        inject(0)
        for t in range(Tn):
            # recurrent matmuls accumulate onto the injected gx, the two
            # scans' chains interleaved pairwise so they run concurrently on
            # col groups 0/1; ScalarE reads gates from PSUM per half-group
            gAc, gBc = gA[0], gB[0]
            for half, cols in ((gAc, (0, 1)), (gBc, (3, 2))):
                for k in range(KH):
                    for n in cols:
                        dst_lo = 512 * (n % 2)
                        for s in scans:
                            nc.tensor.matmul(
                                half[sl(s), dst_lo:dst_lo + 512],
                                hT[s][:, Bl * k:Bl * (k + 1)],
                                wsb[s][:, k, 512 * n:512 * (n + 1)],
                                start=False, stop=(k == KH - 1),
                                skip_group_check=True,
                                tile_position=(0, strip[s]))
            gact = wkpool.tile([48, G], F32, tag="gact", name="gact")
            if t % RC == 0:
                ring[0] = rpool.tile([48, RC, H], BF16, tag="ring", name="ring")
            for s in scans:
                nc.scalar.activation(gact[sl(s), 0:H2], gAc[sl(s), :], AF.Sigmoid)
            for s in scans:
                nc.scalar.activation(gact[sl(s), 3 * H:G], gBc[sl(s), H:H2], AF.Tanh)
            for s in scans:
                nc.scalar.activation(gact[sl(s), H2:3 * H], gBc[sl(s), 0:H], AF.Sigmoid)
            t1 = wkpool.tile([48, H], F32, tag="t1", name="t1")
            t2 = wkpool.tile([48, H], F32, tag="t2", name="t2")
            for s in scans:
                nc.vector.tensor_tensor(out=t1[sl(s), :], in0=gact[sl(s), H:H2],
                                        in1=c_all[sl(s), :], op=ALU.mult)
            for s in scans:
                nc.vector.tensor_tensor(out=t2[sl(s), :], in0=gact[sl(s), 0:H],
                                        in1=gact[sl(s), 3 * H:G], op=ALU.mult)
            for s in scans:
                nc.vector.tensor_tensor(out=c_all[sl(s), :], in0=t1[sl(s), :],
                                        in1=t2[sl(s), :], op=ALU.add)
            tch = wkpool.tile([48, H], F32, tag="tch", name="tch")
            for s in scans:
                nc.scalar.activation(tch[sl(s), :], c_all[sl(s), :], AF.Tanh)
            h16 = wkpool.tile([48, H], BF16, tag="h16", name="h16")
            for s in scans:
                nc.vector.tensor_tensor(out=h16[sl(s), :], in0=gact[sl(s), H2:3 * H],
                                        in1=tch[sl(s), :], op=ALU.mult)
            # allocate transpose psum tiles first (keeps the slot rotation),
            # then emit next step's gx injects BEFORE the transpose matmuls
            # so they fill the PE window spent waiting for h16
            hT_ps = {}
            for s in scans:
                hT_ps[s] = pspool.tile([128, KH * Bl], F32, tag="ps", name="hT_ps")
            if t + 1 < Tn:
                if (t + 1) % TC == 0 and (t + 1) // TC + 1 < nwin:
                    load_gx((t + 1) // TC + 1)
                inject(t + 1)
            for s in scans:
                for k in range(KH):
                    nc.tensor.matmul(hT_ps[s][:, Bl * k:Bl * (k + 1)],
                                     h16[sl(s), 128 * k:128 * (k + 1)],
                                     id_all[sl(s), :], start=True, stop=True,
                                     tile_position=(strip[s], 0))
                hTn = htpool.tile([128, KH * Bl], BF16, tag="hT", name="hTn")
                nc.scalar.activation(hTn[:], hT_ps[s][:], AF.Copy)
                hT[s] = hTn
            for s in scans:
                nc.vector.tensor_scalar_mul(ring[0][sl(s), t % RC, :], h16[sl(s), :],
                                            mask_all[sl(s), t:t + 1])
            if (t + 1) % RC == 0:
                t0r = t + 1 - RC
                for s in scans:
                    nc.gpsimd.dma_start(houtv[s][:, t0r:t0r + RC, :],
                                        ring[0][sl(s), :, :])


def _prep_inputs(inputs, Tn=T, Bl=BL, ncores=NC):
    """Host-side sharding + weight preprocessing. Returns per-core in_maps."""
    x = np.asarray(inputs["x"]).astype(np.int32)
    lengths = np.asarray(inputs["lengths"]).astype(np.int32)
    emb = np.asarray(inputs["emb"], dtype=np.float32)
    ntok = Bl * Tn

    com = {"emb": emb, "ident": np.eye(16, dtype=ml_dtypes.bfloat16)}
    for s in ("f1", "b1", "f2", "b2"):
        w_ih = np.asarray(inputs[f"W_ih_{s}"], np.float32)[_GATE_PERM]
        w_hh = np.asarray(inputs[f"W_hh_{s}"], np.float32)[_GATE_PERM]
        b = np.asarray(inputs[f"b_{s}"], np.float32)[_GATE_PERM]
        com[f"wihT_{s}"] = np.ascontiguousarray(w_ih.T).astype(ml_dtypes.bfloat16)
        com[f"whhT_{s}"] = np.ascontiguousarray(w_hh.T).astype(ml_dtypes.bfloat16)
        com[f"bias_{s}"] = np.tile(b.reshape(1, G), (128, 1))
    com["wclsT"] = np.ascontiguousarray(
        np.asarray(inputs["W_cls"], np.float32).T).astype(ml_dtypes.bfloat16)
    com["bcls"] = np.asarray(inputs["b_cls"], np.float32).reshape(TAGS, 1)

    def chunked(a):  # [ntok] -> [128, ntok//128] with chunk c in column c
        return np.ascontiguousarray(a.reshape(-1).reshape(ntok // 128, 128).T)

    in_maps = []
    for c in range(ncores):
        xs = x[Bl * c:Bl * (c + 1), :Tn]
        ls = np.minimum(lengths[Bl * c:Bl * (c + 1)], Tn)
        ts = np.arange(Tn)[None, :]
        rev = np.where(ts < ls[:, None], ls[:, None] - 1 - ts, ts)  # [Bl,Tn]
        xrev = np.take_along_axis(xs, rev, axis=1)
        flat_rev = (np.arange(Bl)[:, None] * Tn + rev).astype(np.int32)
        m = {
            "xf_idx": chunked(xs),
            "xb_idx": chunked(xrev),
            "rev128": chunked(flat_rev),
            "mask": (ts < ls[:, None]).astype(np.float32),
        }
        m.update(com)
        in_maps.append(m)
    return in_maps


_CACHED = {}


def kernel(**inputs) -> np.ndarray:
    if "nc" not in _CACHED:
        nc = bacc.Bacc("TRN2", target_bir_lowering=False, debug=False,
                       num_devices=NC)
        _build(nc)
        nc.compile()
        _CACHED["nc"] = nc
    nc = _CACHED["nc"]
    in_maps = _prep_inputs(inputs)
    res = run_bass_kernel_spmd(nc, in_maps, core_ids=list(range(NC)), trace=False)
    outs = []
    for c in range(NC):
        lt = res.results[c]["logitsT"]  # [TAGS, ntok]
        outs.append(np.ascontiguousarray(lt.T.reshape(BL, T, TAGS)))
    return np.concatenate(outs, axis=0).astype(np.float32)
